# revision 40
# baseline (speedup 1.0000x reference)
"""MultiHeadAttention forward on 8 Trainium2 NeuronCores (Bass/Tile).

Problem (hardcoded): B=2, S=2048, D=1024, H=16, HD=64.
  qkv = x @ w_qkv.T + b_qkv ; per-head attention with softmax(q k^T/8 + mask);
  out = values @ w_out.T + b_out.

Sharding: tensor-parallel over heads -- core c owns heads {2c, 2c+1}
(value dims 128c..128c+127).  Each core computes its 2 heads end-to-end and
a partial output projection; the host sums the 8 partials (bf16) and adds
the bias constant (b_out + b_v @ w_out.T, exact because softmax rows sum
to 1, and q.bk-type score shifts are softmax-invariant).

Device layout notes:
 - everything bf16 on the PE (same PE rate as f32r, half the DMA/SBUF).
 - scores are computed TRANSPOSED (S^T[k,tq] = K^T.T @ Q^T per head); the
   two heads' score matmuls hit disjoint PE row groups (partitions 0-63 /
   64-127) and run concurrently.
 - AV is emitted as K=64 half-chunks, diagonally paired across heads so
   the two halves also run concurrently on disjoint PE row groups:
   (h0,lo)+(h1,hi) then (h0,hi)+(h1,lo).
 - vext carries 32 ones columns, so AV rows 64..95 hold the softmax
   denominator l; a 32x32 DVE block transpose makes l partition-parallel
   for a cheap reciprocal, a second transpose brings 1/l back as a row,
   and a K=1 PE matmul broadcasts it across partitions for the DVE
   normalize.  (No SBUF->SBUF shuffle DMAs: tiny partition-scatter HWDGE
   transfers were observed to wedge the NeuronCore.)
 - valsT is one [128, T] tile (head h on partitions 64h..64h+63) so the
   output projection is a single K=128 matmul per out-tile.
 - emission is software-pipelined: Phase A for batch 1 and each block's
   output projection are queued as "filler" work and pulled between
   attention chunks, keeping the PE stream dense (p-state!).
 - DMAs are batched: one descriptor-set per 512-token x block and one per
   output block (the startup stall from many tiny DMAs costs ~70us).
"""
import sys
if "/opt/trn_rl_repo" not in sys.path:
    sys.path.insert(0, "/opt/trn_rl_repo")
import numpy as np
from collections import deque

B, S, D, H = 2, 2048, 1024, 16
HD = D // H           # 64
NCORES = 8
T = B * S             # 4096 tokens
NB = S // 512         # 4 tq blocks per batch
NCH = S // 128        # 16 kpos chunks per batch

_CACHE = {}
import os as _os
_BATCH_X = _os.environ.get("BATCH_X", "1") == "1"
_BATCH_OUT = _os.environ.get("BATCH_OUT", "1") == "1"
# K=64 row-paired AV (alternating PE tile configs inside one PSUM
# accumulation group) crashes the NEFF execution on HW -- keep off.
# Custom-DVE ops (reciprocal_approx_fast) and GpSimd ucode ops
# (partition_broadcast) misbehave on this runtime -- plain ops only.
_PAIR_AV = _os.environ.get("PAIR_AV", "0") == "1"
_FILLER = _os.environ.get("FILLER", "1") == "1"


def build_nc(use_mask: bool, reps: int = 1):
    """Build + compile the per-core Bass program (SPMD-identical)."""
    import concourse.bacc as bacc
    import concourse.tile as tile
    from concourse import mybir

    f32 = mybir.dt.float32
    f32r = mybir.dt.float32r
    bf16 = mybir.dt.bfloat16
    EXP = mybir.ActivationFunctionType.Exp
    MULT = mybir.AluOpType.mult

    nc = bacc.Bacc("TRN2", target_bir_lowering=False, debug=False,
                   num_devices=NCORES)

    xTb = nc.dram_tensor("xTb", (D, T), bf16, kind="ExternalInput")
    wqkT = nc.dram_tensor("wqkT", (D, 256), bf16, kind="ExternalInput")
    bqk = nc.dram_tensor("bqk", (128, 2), f32, kind="ExternalInput")
    wvT = nc.dram_tensor("wvT", (D, 128), bf16, kind="ExternalInput")
    woT = nc.dram_tensor("woT", (128, D), bf16, kind="ExternalInput")
    identb = nc.dram_tensor("identb", (128, 128), bf16, kind="ExternalInput")
    if use_mask:
        maskT = nc.dram_tensor("maskT", (B, S, S), f32r, kind="ExternalInput")
        ident = nc.dram_tensor("ident", (128, 128), f32r, kind="ExternalInput")
    out = nc.dram_tensor("out", (T, D), bf16, kind="ExternalOutput")

    with tile.TileContext(nc) as tc:
        with tc.tile_pool(name="sbp", bufs=1) as sbp, \
             tc.tile_pool(name="xtbp", bufs=2) as xtbp, \
             tc.tile_pool(name="ptp", bufs=4) as ptp, \
             tc.tile_pool(name="lrp", bufs=2) as lrp, \
             tc.tile_pool(name="otp", bufs=2) as otp, \
             tc.tile_pool(name="mkp", bufs=4) as mkp, \
             tc.tile_pool(name="mmp", bufs=2, space="PSUM") as mmp, \
             tc.tile_pool(name="scp", bufs=2, space="PSUM") as scp, \
             tc.tile_pool(name="avp", bufs=2, space="PSUM") as avp:

            # --- persistent SBUF tensors ---
            qkt = sbp.tile([128, 2, T], bf16, name="qkt")    # [feat,{q,k},tok]
            vext = sbp.tile([128, B, 2, NCH, HD + 32], bf16, name="vext")
            vT_sb = sbp.tile([128, T], bf16, name="vT_sb")   # [vfeat, tok]
            valsT = sbp.tile([128, T], bf16, name="valsT")   # [64h+d, tok]
            wqk_sb = sbp.tile([128, 8, 256], bf16, name="wqk_sb")
            wv_sb = sbp.tile([128, 8, 128], bf16, name="wv_sb")
            wo_sb = sbp.tile([128, D], bf16, name="wo_sb")
            bqk_sb = sbp.tile([128, 2], f32, name="bqk_sb")
            idb_sb = sbp.tile([128, 128], bf16, name="idb_sb")
            ones_sb = sbp.tile([65, 64], f32, name="ones_sb")
            nc.vector.memset(ones_sb, 1.0)
            if use_mask:
                id_sb = sbp.tile([128, 128], f32r, name="id_sb")
                nc.sync.dma_start(id_sb, ident[:, :])

            nc.sync.dma_start(
                wqk_sb, wqkT[:, :].rearrange("(c p) j -> p c j", p=128))
            nc.sync.dma_start(
                wv_sb, wvT[:, :].rearrange("(c p) j -> p c j", p=128))
            nc.sync.dma_start(wo_sb, woT[:, :])
            nc.sync.dma_start(bqk_sb, bqk[:, :])
            nc.sync.dma_start(idb_sb, identb[:, :])
            nc.vector.memset(vext[:, :, :, :, HD:HD + 32], 1.0)

            def phase_a_block(rep, b, tb):
                """Emit the list of closures for one 512-token projection
                block (tb in 0..7 global).  Returned items are emitted lazily
                as filler."""
                items = []
                xtb_t = xtbp.tile([128, 8, 512], bf16, tag="xtb",
                                  name=f"xtb_{rep}_{tb}")

                def load():
                    if _BATCH_X:
                        if tb == 0:
                            # split the very first load so the first qk
                            # matmuls start after half the transfer
                            for lo in (0, 4):
                                src = xTb[128 * lo:128 * lo + 512,
                                          0:512].rearrange(
                                    "(c p) j -> p c j", p=128)
                                nc.sync.dma_start(xtb_t[:, lo:lo + 4, :], src)
                        else:
                            src = xTb[:, 512 * tb:512 * tb + 512].rearrange(
                                "(c p) j -> p c j", p=128)
                            nc.sync.dma_start(xtb_t, src)
                    else:
                        for c in range(8):
                            nc.sync.dma_start(
                                xtb_t[:, c, :],
                                xTb[128 * c:128 * c + 128,
                                    512 * tb:512 * tb + 512])
                items.append(load)

                for m in range(2):          # q then k projections
                    def qk(m=m):
                        acc = mmp.tile([128, 512], f32, tag="mm",
                                       name=f"qk_{rep}_{tb}_{m}")
                        for c in range(8):
                            nc.tensor.matmul(
                                acc, wqk_sb[:, c, 128 * m:128 * m + 128],
                                xtb_t[:, c, :], start=(c == 0), stop=(c == 7))
                        nc.vector.tensor_scalar_add(
                            qkt[:, m, 512 * tb:512 * tb + 512], acc,
                            bqk_sb[:, m:m + 1])
                    items.append(qk)

                def vpass():
                    # v^T [vfeat, tok] with wv stationary (long streams,
                    # weight loads hidden), then cast to bf16 SBUF.
                    vacc = mmp.tile([128, 512], f32, tag="mm",
                                    name=f"vacc_{rep}_{tb}")
                    for c in range(8):
                        nc.tensor.matmul(
                            vacc, wv_sb[:, c, :], xtb_t[:, c, :],
                            start=(c == 0), stop=(c == 7))
                    nc.vector.tensor_copy(
                        vT_sb[:, 512 * tb:512 * tb + 512], vacc)
                items.append(vpass)

                for u in range(4):          # transpose to [kpos, feat] tiles
                    def vtrans(u=u):
                        t0g = 512 * tb + 128 * u
                        cc = (t0g % S) // 128
                        vtp = mmp.tile([128, 128], f32, tag="mm",
                                       name=f"vtp_{rep}_{tb}_{u}")
                        nc.tensor.matmul(vtp, vT_sb[:, t0g:t0g + 128], idb_sb,
                                         start=True, stop=True)
                        nc.vector.tensor_copy(
                            vext[:, b, :, cc, 0:HD],
                            vtp[:, :].rearrange("p (h d) -> p h d", h=2))
                    items.append(vtrans)
                return items

            def pull(filler, n):
                for _ in range(n):
                    if not filler:
                        return
                    filler.popleft()()

            def attention_block(rep, b, tqb, filler):
                tq0 = S * b + 512 * tqb
                q_aps = [qkt[64 * h:64 * h + 64, 0, tq0:tq0 + 512]
                         for h in range(2)]
                avs = [avp.tile([96, 512], f32, tag="av",
                                name=f"av_{rep}_{b}_{tqb}_{h}")
                       for h in range(2)]
                def emit_scores(c):
                    sc = scp.tile([128, 1024], f32, tag="sc",
                                  name=f"sc_{rep}_{b}_{tqb}_{c}")
                    for h in range(2):
                        k_ap = qkt[64 * h:64 * h + 64, 1,
                                   S * b + 128 * c:S * b + 128 * c + 128]
                        nc.tensor.matmul(
                            sc[:, 512 * h:512 * h + 512], k_ap, q_aps[h],
                            start=True, stop=(not use_mask))
                    if use_mask:
                        mt = mkp.tile([128, 512], f32r, tag="mk",
                                      name=f"mk_{rep}_{b}_{tqb}_{c}")
                        nc.sync.dma_start(
                            mt, maskT[b, 128 * c:128 * c + 128,
                                      512 * tqb:512 * tqb + 512])
                        for h in range(2):
                            nc.tensor.matmul(
                                sc[:, 512 * h:512 * h + 512], id_sb, mt,
                                start=False, stop=True)
                    return sc

                # Software-pipeline the scores one chunk ahead of AV: the
                # in-order PE queue would otherwise head-block scores(c+1)
                # behind AV(c)'s wait on exp(c), stalling ScalarE each chunk.
                sc_cur = emit_scores(0)
                for c in range(NCH):
                    sc_next = emit_scores(c + 1) if c + 1 < NCH else None
                    pull(filler, 2)
                    pt = ptp.tile([128, 1024], bf16, tag="pt",
                                  name=f"pt_{rep}_{b}_{tqb}_{c}")
                    nc.scalar.activation(pt, sc_cur, EXP)
                    for h in range(2):
                        nc.tensor.matmul(
                            avs[h], vext[:, b, h, c, :],
                            pt[:, 512 * h:512 * h + 512],
                            start=(c == 0), stop=(c == NCH - 1))
                    sc_cur = sc_next
                # --- normalize: valsT[64h:, tq] = av[0:64] / l ---
                # av rows 64..95 all hold l (32 ones columns in vext);
                # 32x32 DVE block transposes give a partition-parallel
                # reciprocal, then a K=1 PE matmul broadcasts 1/l across
                # partitions for the DVE multiply.
                for h in range(2):
                    av = avs[h]
                    # copy av out of PSUM first so the (doubly-scarce) av
                    # buffer frees for the next block's AV immediately,
                    # instead of after the whole reciprocal chain.
                    av_sb = lrp.tile([64, 512], f32, tag="avs",
                                     name=f"avs_{rep}_{b}_{h}_{tqb}")
                    nc.vector.tensor_copy(av_sb, av[0:64, :])
                    ls = lrp.tile([96, 512], f32, tag="ls",
                                  name=f"ls_{rep}_{b}_{h}_{tqb}")
                    nc.vector.tensor_copy(ls[64:96, :], av[64:96, :])
                    lt = lrp.tile([96, 512], f32, tag="lt",
                                  name=f"lt_{rep}_{b}_{h}_{tqb}")
                    nc.vector.transpose(lt[64:96, :], ls[64:96, :])
                    lt3 = lt[64:96, :].rearrange(
                        "p (a b) -> p a b", b=32)[:, :, 0:1]
                    nc.vector.reciprocal(lt3, lt3)
                    rlrowf = lrp.tile([96, 512], f32, tag="rlrowf",
                                      name=f"rlrowf_{rep}_{b}_{h}_{tqb}")
                    nc.vector.transpose(rlrowf[64:96, :], lt[64:96, :])
                    rlrow = lrp.tile([65, 512], f32r, tag="rlrow",
                                     name=f"rlrow_{rep}_{b}_{h}_{tqb}")
                    nc.vector.tensor_copy(rlrow[64:65, :], rlrowf[64:65, :])
                    bcp = mmp.tile([64, 512], f32, tag="mm",
                                   name=f"bcp_{rep}_{b}_{h}_{tqb}")
                    nc.tensor.matmul(
                        bcp, ones_sb[64:65, :].bitcast(f32r),
                        rlrow[64:65, :], start=True, stop=True)
                    bcs = lrp.tile([64, 512], f32, tag="bcs",
                                   name=f"bcs_{rep}_{b}_{h}_{tqb}")
                    nc.vector.tensor_copy(bcs, bcp)
                    nc.vector.tensor_tensor(
                        valsT[64 * h:64 * h + 64, tq0:tq0 + 512],
                        av_sb, bcs, MULT)
                # --- queue this block's output projection as filler ---
                ot = otp.tile([128, 4, 2, 512], bf16, tag="ot",
                              name=f"ot_{rep}_{b}_{tqb}")
                for u in range(4):
                    for nb2 in range(2):
                        def op_item(u=u, nb2=nb2):
                            t0 = tq0 + 128 * u
                            op = mmp.tile([128, 512], f32, tag="mm",
                                          name=f"op_{rep}_{b}_{tqb}_{nb2}_{u}")
                            nc.tensor.matmul(
                                op, valsT[:, t0:t0 + 128],
                                wo_sb[:, 512 * nb2:512 * nb2 + 512],
                                start=True, stop=True)
                            nc.vector.tensor_copy(ot[:, u, nb2, :], op)
                        filler.append(op_item)

                def out_dma():
                    if _BATCH_OUT:
                        dst = out[tq0:tq0 + 512, :].rearrange(
                            "(u p) (nb j) -> p u nb j", p=128, j=512)
                        nc.sync.dma_start(dst, ot)
                    else:
                        for u in range(4):
                            t0 = tq0 + 128 * u
                            nc.sync.dma_start(
                                out[t0:t0 + 128, :].rearrange(
                                    "p (nb j) -> p nb j", j=512),
                                ot[:, u, :, :])
                filler.append(out_dma)

            for rep in range(reps):
                filler = deque()
                # Only tb0 of Phase A is emitted directly: attention(b0,
                # tqb0) needs just the first k/v chunks, so tb1-3 stream in
                # as filler during its chunk loop (chunk 4c needs tb c,
                # pulled 2 items/chunk -> arrives just in time).  Batch 1's
                # blocks become filler for the later b0 attention blocks.
                for item in phase_a_block(rep, 0, 0):
                    item()
                if _FILLER:
                    for tb in range(1, 4):
                        filler.extend(phase_a_block(rep, 0, tb))
                else:
                    for tb in range(1, 8):
                        for item in phase_a_block(rep, tb // 4, tb):
                            item()
                for i, (b, tqb) in enumerate(
                        [(b, t) for b in range(B) for t in range(NB)]):
                    attention_block(rep, b, tqb, filler)
                    if _FILLER and b == 0:
                        filler.extend(phase_a_block(rep, 1, 4 + tqb))
                    if not _FILLER:
                        while filler:
                            filler.popleft()()
                while filler:
                    filler.popleft()()
    nc.compile()
    return nc


def make_in_maps(mha_x, self_mask, w_qkv, b_qkv, w_out, b_out, use_mask):
    """Host-side sharding / layout prep. Returns (in_maps, host_bias)."""
    import ml_dtypes
    bf = np.dtype(ml_dtypes.bfloat16)
    x = np.asarray(mha_x, np.float32).reshape(T, D)
    xTb_np = np.ascontiguousarray(x.T.astype(bf))       # [D, T]
    scale = 1.0 / np.sqrt(np.float32(HD))               # 1/8
    wqkv = np.asarray(w_qkv, np.float32)
    bqkv = np.asarray(b_qkv, np.float32)
    wout = np.asarray(w_out, np.float32)
    bout = np.asarray(b_out, np.float32)

    # reference packs w_qkv rows as [H, (q,k,v), HD]: head h's q rows are
    # wqkv[192h:192h+64], k rows +64, v rows +128.
    wq_rows = lambda h: wqkv[192 * h:192 * h + 64, :]
    wk_rows = lambda h: wqkv[192 * h + 64:192 * h + 128, :]
    wv_rows = lambda h: wqkv[192 * h + 128:192 * h + 192, :]
    bq_of = lambda h: bqkv[192 * h:192 * h + 64]
    bk_of = lambda h: bqkv[192 * h + 64:192 * h + 128]
    bv_of = lambda h: bqkv[192 * h + 128:192 * h + 192]

    in_maps = []
    for c in range(NCORES):
        h0, h1 = 2 * c, 2 * c + 1
        wq = np.concatenate([wq_rows(h0), wq_rows(h1)], 0) * scale
        wk = np.concatenate([wk_rows(h0), wk_rows(h1)], 0)
        wv = np.concatenate([wv_rows(h0), wv_rows(h1)], 0)
        m = {
            "xTb": xTb_np,
            "wqkT": np.ascontiguousarray(
                np.concatenate([wq, wk], 0).T.astype(bf)),
            "bqk": np.ascontiguousarray(
                np.stack([np.concatenate([bq_of(h0), bq_of(h1)]) * scale,
                          np.concatenate([bk_of(h0), bk_of(h1)])], 1)),
            "wvT": np.ascontiguousarray(wv.T.astype(bf)),
            "woT": np.ascontiguousarray(
                wout[:, 128 * c:128 * c + 128].T.astype(bf)),
            "identb": np.eye(128, dtype=np.float32).astype(bf),
        }
        if use_mask:
            m["maskT"] = np.ascontiguousarray(
                np.asarray(self_mask, np.float32).transpose(0, 2, 1))
            m["ident"] = np.eye(128, dtype=np.float32)
        in_maps.append(m)

    b_v_full = np.concatenate([bv_of(h) for h in range(H)])
    host_bias = b_v_full @ wout.T + bout                # [D], exact
    return in_maps, host_bias


def kernel(**inputs):
    from concourse.bass_utils import run_bass_kernel_spmd
    self_mask = np.asarray(inputs["self_mask"], np.float32)
    use_mask = bool(np.any(self_mask))
    key = ("nc", use_mask)
    if key not in _CACHE:
        _CACHE[key] = build_nc(use_mask)
    nc = _CACHE[key]
    in_maps, host_bias = make_in_maps(
        inputs["mha_x"], self_mask, inputs["w_qkv"], inputs["b_qkv"],
        inputs["w_out"], inputs["b_out"], use_mask)
    res = run_bass_kernel_spmd(nc, in_maps, core_ids=list(range(NCORES)))
    acc = np.zeros((T, D), np.float32)
    for c in range(NCORES):
        acc += res.results[c]["out"].astype(np.float32)
    acc += host_bias[None, :]
    return acc.reshape(B, S, D)
